# revision 1
# baseline (speedup 1.0000x reference)
"""GNN message-passing kernel for 8 Trainium2 NeuronCores.

Reference computation:
    t   = node_feats @ W + b                       # [N, H]
    msgs = t[nbr] + edge_feats[eid]                # [E, H]
    agg = segment_sum(msgs, dst, N)                # [N, H]
    out = t + agg

Sharding: dst is sorted, so core k owns the node range [k*6250, (k+1)*6250)
and the contiguous edge slice whose dst falls in that range. No cross-core
combination of outputs is needed; only the node transform t~ = node_feats @ W
(bias folded out) is shared, via one fp8 AllGather.

Per core, edges are grouped into 49 windows of 128 dst nodes. Each window
accumulates in one PSUM tile [128, 256] via one-hot matmuls:
  psum += I.T @ t_own(window)                      (identity matmul, bf16)
  psum += (1+deg).T @ b                            (K=1 outer product)
  psum += onehot(tile).T @ g(tile)                 (neighbor term, fp8 x fp8)
  psum += onehot(tile).T @ ef(tile)                (edge-feat term, fp8 x fp8)
where g = fp8 rows of t~ gathered with gpsimd.dma_gather, ef = fp8 edge
features staged slot-aligned on the host, and the fp8 one-hot is prebuilt on
the host and streamed (pad slots have an all-zero one-hot row).

The gather table is split in two (A: shard offset < 4096 -> 32768 rows,
B: rest -> 17232 rows) so gather indices fit int16. Pad slots carry index
-1 and each gather's count register is the exact per-core slot count, so
pads cost no descriptors or HBM traffic; the fixed-size gather tiles are
zeroed once up front so untouched pad columns stay finite.

All 8-bit tensors use e3m4 (4 mantissa bits): quantizing the gathered
t-rows and edge features at ~0.6% rms each puts the output at ~1.3e-2
relative error against the f32 reference (gate: 2e-2).

This version attacks the HBM/DMA roofline: the baseline streamed f32 edge
features plus a bf16 one-hot (~220 MB/core); this streams fp8 edge
features + fp8 one-hots and gathers fp8 rows (~70 MB/core).
"""

import sys

sys.path.insert(0, "/opt/trn_rl_repo")

import ml_dtypes
import numpy as np

import concourse.bacc as bacc
import concourse.mybir as mybir
import concourse.tile as tile
from concourse.bass_utils import run_bass_kernel_spmd
from concourse.library_config import mlp

N_NODES = 50000
N_EDGES = 800000
H = 256
N_CORES = 8
NODES_PER_CORE = N_NODES // N_CORES          # 6250
WIN = 128                                    # dst nodes per PSUM window
N_WIN = (NODES_PER_CORE + WIN - 1) // WIN    # 49 (last window = 106 nodes)
OFF_A = 4096                                 # per-rank offset split: A = off<4096 (32 windows)
ROWS_A = OFF_A * N_CORES                     # 32768 rows in table A
OFF_B = NODES_PER_CORE - OFF_A               # 2154 rows per rank in table B

F8 = mybir.dt.float8e3                       # e3m4: 4 mantissa bits, range +-15.5
F8NP = ml_dtypes.float8_e3m4

_cache = {}
_last_in_maps = None


def _build_schedule(dst, nbr):
    """Host-side slot schedule, shared across cores (single NEFF).

    Returns (T, c0, S, per_core) where T[(w, q)] is the tile count of window
    w's group-q (gather table A/B), c0[(w, q)] its starting slot-column, S
    the total slot-columns, and per_core[k] the edge->slot assignment arrays.
    """
    bounds = np.searchsorted(dst, np.arange(N_CORES + 1) * NODES_PER_CORE)
    counts = np.zeros((N_CORES, N_WIN, 2), dtype=np.int64)
    per_core_raw = []
    for k in range(N_CORES):
        e0, e1 = bounds[k], bounds[k + 1]
        dk = dst[e0:e1].astype(np.int64) - k * NODES_PER_CORE
        nk = nbr[e0:e1].astype(np.int64)
        w = dk >> 7
        q = ((nk % NODES_PER_CORE) >= OFF_A).astype(np.int64)
        key = w * 2 + q
        np.add.at(counts[k], (w, q), 1)
        per_core_raw.append((e0, e1, dk, nk, key))

    T = np.maximum((counts.max(axis=0) + 127) // 128, 1)   # [N_WIN, 2] tiles
    c0 = np.zeros((N_WIN, 2), dtype=np.int64)
    s = 0
    for w in range(N_WIN):
        for q in range(2):
            c0[w, q] = s
            s += T[w, q]
    S = int(s)

    per_core = []
    for k in range(N_CORES):
        e0, e1, dk, nk, key = per_core_raw[k]
        # secondary sort by nbr so each group's slots gather ascending addresses
        order = np.lexsort((nk, key))
        sorted_key = key[order]
        group_start = np.searchsorted(sorted_key, np.arange(N_WIN * 2))
        j_within = np.arange(len(order)) - group_start[sorted_key]
        slot = np.empty(len(order), dtype=np.int64)
        slot[order] = j_within
        kw = key >> 1
        kq = key & 1
        base_col = c0[kw, kq]
        p = slot % 128
        c = base_col + slot // 128
        per_core.append((e0, e1, dk, nk, p, c))
    return T, c0, S, per_core


def _build_program(T, c0, S):
    """Build + schedule the shared SPMD program for slot schedule (T, S)."""
    nc = bacc.Bacc(num_swdge_queues=4)
    f32, bf16, i16 = mybir.dt.float32, mybir.dt.bfloat16, mybir.dt.int16

    i32 = mybir.dt.int32
    nfT = nc.declare_dram_parameter("nfT", [H, NODES_PER_CORE], bf16, isOutput=False)
    W16 = nc.declare_dram_parameter("W16", [H, H], bf16, isOutput=False)
    b16p = nc.declare_dram_parameter("b16", [1, H], bf16, isOutput=False)
    deg1 = nc.declare_dram_parameter("deg1", [1, N_WIN * WIN], bf16, isOutput=False)
    identp = nc.declare_dram_parameter("ident", [128, 128], bf16, isOutput=False)
    ef = nc.declare_dram_parameter("ef", [128, S, H], F8, isOutput=False)
    ohp = nc.declare_dram_parameter("oh", [128, S * 128], F8, isOutput=False)
    cntp = nc.declare_dram_parameter("cnt", [1, 2 * N_WIN], i32, isOutput=False)
    gidx = nc.declare_dram_parameter("gidx", [128, 8 * S], i16, isOutput=False)
    outp = nc.declare_dram_parameter("out", [NODES_PER_CORE, H], f32, isOutput=True)
    twmax = int(T.max())

    townA = nc.dram_tensor("townA", [OFF_A, H], F8)
    townB = nc.dram_tensor("townB", [OFF_B, H], F8)
    tfullA = nc.dram_tensor("tfullA", [OFF_A * N_CORES, H], F8, addr_space="Shared")
    tfullB = nc.dram_tensor("tfullB", [OFF_B * N_CORES, H], F8, addr_space="Shared")

    with tile.TileContext(nc) as tc:
        nc.gpsimd.load_library(mlp)
        with (
            tc.tile_pool(name="const", bufs=1) as cpool,
            tc.tile_pool(name="psum", bufs=2, space="PSUM") as pp,
            tc.tile_pool(name="wpsum", bufs=6, space="PSUM") as wpp,
            tc.tile_pool(name="gath", bufs=8) as gp,
            tc.tile_pool(name="ef8", bufs=8) as e8p,
            tc.tile_pool(name="oneh", bufs=8) as ohpool,
            tc.tile_pool(name="flush", bufs=3) as flp,
        ):
            # --- resident constants; nfT stage-A chunks load first so the
            # phase-1 matmuls can start as early as possible ---------------
            w16 = cpool.tile([128, 2 * H], bf16)           # W in two K-halves
            nc.gpsimd.dma_start(out=w16[:, :H], in_=W16[0:128, :])
            nc.gpsimd.dma_start(out=w16[:, H:], in_=W16[128:256, :])
            b16 = cpool.tile([1, H], bf16)
            nc.gpsimd.dma_start(out=b16[:], in_=b16p[:])
            d16 = cpool.tile([1, N_WIN * WIN], bf16)
            nc.gpsimd.dma_start(out=d16[:], in_=deg1[:])
            id16 = cpool.tile([128, 128], bf16)
            nc.gpsimd.dma_start(out=id16[:], in_=identp[:])
            cnt_s = cpool.tile([1, 2 * N_WIN], i32)
            nc.gpsimd.dma_start(out=cnt_s[:], in_=cntp[:])
            gidx_s = cpool.tile([128, 8 * S], i16)
            nc.sync.dma_start(out=gidx_s[:], in_=gidx[:])
            warm = nc.dram_tensor("warm", [1, 128], bf16)
            warm_out = nc.dram_tensor("warm_out", [N_CORES, 128], bf16, addr_space="Shared")
            nc.gpsimd.collective_compute(
                "AllGather", mybir.AluOpType.bypass,
                replica_groups=[list(range(N_CORES))],
                ins=[warm[:]], outs=[warm_out[:]],
            )
            town = cpool.tile([128, N_WIN * H], bf16)      # own t~, node i%128 / col i//128
            town8 = cpool.tile([128, N_WIN * H], F8)       # fp8 copy for the gather table

            # --- phase 1: own t~ shard in two stages, pipelined AllGathers
            nf16 = cpool.tile([128, 2 * NODES_PER_CORE], bf16)
            W_A = OFF_A // WIN                             # 32 windows in stage A
            for stage in range(2):
                lo = 0 if stage == 0 else OFF_A
                hi = OFF_A if stage == 0 else NODES_PER_CORE
                mid = (lo + hi) // 2
                for a, b in ((lo, mid), (mid, hi)):
                    nc.gpsimd.dma_start(out=nf16[:, a:b], in_=nfT[0:128, a:b])
                    nc.gpsimd.dma_start(out=nf16[:, NODES_PER_CORE + a:NODES_PER_CORE + b], in_=nfT[128:256, a:b])
                for i in range(lo // WIN, (hi + WIN - 1) // WIN):
                    n0 = i * WIN
                    nn = min(WIN, NODES_PER_CORE - n0)
                    ps = pp.tile([128, H], f32, tag="ph1ps")
                    nc.tensor.matmul(ps[:nn, :], lhsT=nf16[:, n0:n0 + nn], rhs=w16[:, :H], start=True, stop=False)
                    nc.tensor.matmul(ps[:nn, :], lhsT=nf16[:, NODES_PER_CORE + n0:NODES_PER_CORE + n0 + nn], rhs=w16[:, H:], start=False, stop=True)
                    nc.vector.tensor_copy(out=town[:nn, i * H:(i + 1) * H], in_=ps[:nn, :])
                    nc.scalar.copy(out=town8[:nn, i * H:(i + 1) * H], in_=ps[:nn, :])
                if stage == 0:
                    nc.sync.dma_start(
                        out=townA[:].rearrange("(w p) f -> p w f", p=128),
                        in_=town8[:, :W_A * H].rearrange("p (w f) -> p w f", f=H),
                    )
                    nc.gpsimd.collective_compute(
                        "AllGather", mybir.AluOpType.bypass,
                        replica_groups=[list(range(N_CORES))],
                        ins=[townA[:]], outs=[tfullA[:]],
                    )
                else:
                    nb_full = ((NODES_PER_CORE - OFF_A) // WIN) * WIN  # 2048
                    nc.sync.dma_start(
                        out=townB[:nb_full, :].rearrange("(w p) f -> p w f", p=128),
                        in_=town8[:, W_A * H:(W_A + nb_full // WIN) * H].rearrange("p (w f) -> p w f", f=H),
                    )
                    nc.sync.dma_start(
                        out=townB[nb_full:, :],
                        in_=town8[:OFF_B - nb_full, (W_A + nb_full // WIN) * H:],
                    )
                    nc.gpsimd.collective_compute(
                        "AllGather", mybir.AluOpType.bypass,
                        replica_groups=[list(range(N_CORES))],
                        ins=[townB[:]], outs=[tfullB[:]],
                    )

            # --- phase 2: windows; B-table work lags A by LAG windows ----
            LAG = 4
            psums = {}

            # zero the gather pool once so pad columns (never written after
            # pad-skip) hold finite fp8 values rather than NaN bit patterns
            for _ in range(8):
                z = gp.tile([128, twmax * H], F8, tag="gath")
                nc.vector.memset(z[:], 0.0)

            creg = nc.gpsimd.alloc_register("gather_cnt")

            def group_part(w, q, ps, is_last):
                qn = (2 * w + q) % 4
                tw = int(T[w, q])
                cc = int(c0[w, q])
                g = gp.tile([128, twmax * H], F8, tag="gath")
                src = tfullA[:] if q == 0 else tfullB[:]
                nc.gpsimd.reg_load(creg, cnt_s[0:1, 2 * w + q:2 * w + q + 1])
                nc.gpsimd.dma_gather(
                    out_ap=g[:, :tw * H].rearrange("p (c d) -> p c d", d=H),
                    in_ap=src,
                    idxs_ap=gidx_s[:, 8 * cc: 8 * (cc + tw)],
                    num_idxs=tw * 128,
                    num_idxs_reg=creg,
                    elem_size=H,
                    single_packet=False,
                    queue_num=qn,
                )
                e8 = e8p.tile([128, tw * H], F8, tag="ef8")
                nc.sync.dma_start(out=e8[:].rearrange("p (c d) -> p c d", d=H), in_=ef[:, cc:cc + tw, :])
                oh = ohpool.tile([128, tw * 128], F8, tag="oneh")
                nc.sync.dma_start(out=oh[:], in_=ohp[:, cc * 128:(cc + tw) * 128])
                for cidx in range(tw):
                    ohc = oh[:, cidx * 128:(cidx + 1) * 128]
                    nc.tensor.matmul(ps[:], lhsT=ohc, rhs=g[:, cidx * H:(cidx + 1) * H],
                                     start=False, stop=False)
                    nc.tensor.matmul(ps[:], lhsT=ohc, rhs=e8[:, cidx * H:(cidx + 1) * H],
                                     start=False, stop=is_last and cidx == tw - 1)

            for step in range(N_WIN + LAG):
                if step < N_WIN:
                    w = step
                    n0 = w * WIN
                    nn = min(WIN, NODES_PER_CORE - n0)
                    ps = wpp.tile([128, H], f32, tag="winps")
                    psums[w] = ps
                    nc.tensor.matmul(ps[:nn, :], lhsT=id16[:nn, :nn], rhs=town[:nn, w * H:(w + 1) * H], start=True, stop=False)
                    nc.tensor.matmul(ps[:], lhsT=d16[:, n0:n0 + WIN], rhs=b16[:], start=False, stop=False)
                    group_part(w, 0, ps, False)
                if step >= LAG:
                    w = step - LAG
                    n0 = w * WIN
                    nn = min(WIN, NODES_PER_CORE - n0)
                    ps = psums.pop(w)
                    group_part(w, 1, ps, True)
                    fl = flp.tile([128, H], f32, tag="flush")
                    nc.scalar.copy(out=fl[:nn, :], in_=ps[:nn, :])
                    nc.sync.dma_start(out=outp[n0:n0 + nn, :], in_=fl[:nn, :])
    nc.compile()
    return nc


def kernel(node_feats, edge_feats, W, b, dst, nbr, eid):
    global _last_in_maps
    node_feats = np.ascontiguousarray(np.asarray(node_feats, dtype=np.float32))
    edge_feats = np.ascontiguousarray(np.asarray(edge_feats, dtype=np.float32))
    W = np.ascontiguousarray(np.asarray(W, dtype=np.float32))
    b = np.asarray(b, dtype=np.float32).reshape(1, H)
    dst = np.asarray(dst, dtype=np.int32)
    nbr = np.asarray(nbr, dtype=np.int32)
    eid = np.asarray(eid, dtype=np.int32)

    T, c0, S, per_core = _build_schedule(dst, nbr)

    key = (S, T.tobytes())
    if key not in _cache:
        _cache.clear()
        _cache[key] = _build_program(T, c0, S)
    nc = _cache[key]

    bf = ml_dtypes.bfloat16
    ident = np.eye(128, dtype=bf)
    W16 = W.astype(bf)
    b16 = b.astype(bf)

    in_maps = []
    for k in range(N_CORES):
        e0, e1, dk, nk, p, c = per_core[k]
        ef_arr = np.zeros((128, S, H), dtype=F8NP)
        ef_arr[p, c] = edge_feats[eid[e0:e1]].astype(F8NP)
        oh_arr = np.zeros((128, S, 128), dtype=F8NP)
        oh_arr[p, c, dk & 127] = F8NP(1.0)
        # gather indices: slot-within-group j = (c - group_base_col)*128 + p;
        # index slot j lives at [j%16, 8*group_base_col + j//16]; unused
        # trailing slots stay -1 so the count-register gather skips them
        gidx_arr = np.full((16, 8 * S), -1, dtype=np.int16)
        w_arr = dk >> 7
        rank = nk // NODES_PER_CORE
        off = nk % NODES_PER_CORE
        q_arr = (off >= OFF_A).astype(np.int64)
        idx_val = np.where(q_arr == 0, rank * OFF_A + off, rank * OFF_B + off - OFF_A)
        base_col = c0[w_arr, q_arr]
        j = (c - base_col) * 128 + p
        gidx_arr[j % 16, 8 * base_col + j // 16] = idx_val.astype(np.int16)
        gidx_full = np.tile(gidx_arr, (8, 1))
        cnt_arr = np.zeros((1, 2 * N_WIN), dtype=np.int32)
        np.add.at(cnt_arr[0], w_arr * 2 + q_arr, 1)
        deg1_arr = np.zeros((1, N_WIN * WIN), dtype=np.float32)
        deg1_arr[0, :NODES_PER_CORE] = 1.0
        np.add.at(deg1_arr[0], dk, 1.0)
        nfT_k = np.ascontiguousarray(
            node_feats[k * NODES_PER_CORE:(k + 1) * NODES_PER_CORE].T
        ).astype(bf)
        in_maps.append({
            "nfT": nfT_k,
            "W16": W16,
            "b16": b16,
            "deg1": deg1_arr.astype(bf),
            "ident": ident,
            "ef": ef_arr,
            "oh": oh_arr.reshape(128, S * 128),
            "cnt": cnt_arr,
            "gidx": gidx_full,
        })

    _last_in_maps = in_maps
    res = run_bass_kernel_spmd(nc, in_maps, list(range(N_CORES)))
    out = np.concatenate([res.results[k]["out"] for k in range(N_CORES)], axis=0)
    return out



# revision 12
# speedup vs baseline: 1.4293x; 1.4293x over previous
"""GNN message-passing kernel for 8 Trainium2 NeuronCores.

Reference computation:
    t   = node_feats @ W + b                       # [N, H]
    msgs = t[nbr] + edge_feats[eid]                # [E, H]
    agg = segment_sum(msgs, dst, N)                # [N, H]
    out = t + agg

Sharding: dst is sorted, so core k owns the node range [k*6250, (k+1)*6250)
and the contiguous edge slice whose dst falls in that range. No collectives:
by linearity, sum_e t[nbr_e] = (sum_e nf[nbr_e]) @ W + deg*b, so each core
gathers raw fp8 node features from a replicated table and applies W once per
window after aggregation:
    out = (nf_own + sum nf[nbr]) @ W + (1 + deg) * b + sum ef

Per core, edges are grouped into 49 windows of 128 dst nodes, each split in
two gather groups by neighbor id (A: nbr < 25000, B: rest) so gather indices
fit int16. Slots within a group are nbr-sorted and dense; trailing pad slots
carry index -1 and the per-core count register trims them.

Each window accumulates two PSUM tiles:
  psT[f, d]  (two 128-row halves) += g_tile[:, f_half].T @ onehot    (fp8)
                                   += self_tile[:, f_half].T @ I     (fp8)
  pso[d, h]  += onehot.T @ ef_tile                                   (fp8)
             += (1+deg).T @ b                                        (K=1)
then the W-transform: copy psT to SBUF bf16 and
  pso += S.T @ W  (two K=128 bf16 matmuls), flush pso to HBM.

The one-hot (pad rows all-zero, so garbage in count-trimmed gather slots is
multiplied by zero; the gather pool is also zeroed once so no NaNs appear),
edge features and gather indices are host-built and streamed; all 8-bit
tensors are e3m4.
"""

import sys

sys.path.insert(0, "/opt/trn_rl_repo")

import ml_dtypes
import numpy as np

import concourse.bacc as bacc
import concourse.mybir as mybir
import concourse.tile as tile
from concourse.bass_utils import run_bass_kernel_spmd
from concourse.library_config import mlp

N_NODES = 50000
N_EDGES = 800000
H = 256
N_CORES = 8
NODES_PER_CORE = N_NODES // N_CORES          # 6250
WIN = 128                                    # dst nodes per PSUM window
N_WIN = (NODES_PER_CORE + WIN - 1) // WIN    # 49 (last window = 106 nodes)
SPLIT = 25000                                # A: nbr < SPLIT, B: rest

F8 = mybir.dt.float8e3                       # e3m4: 4 mantissa bits
F8NP = ml_dtypes.float8_e3m4

_cache = {}
_last_in_maps = None


def _build_schedule(dst, nbr):
    """Host-side slot schedule, shared shapes across cores (single NEFF).

    Returns (T, c0, S, per_core): T[(w, q)] tile count of window w group q,
    c0[(w, q)] its starting slot-column, S total slot-columns, per_core[k]
    the per-core edge->slot assignment.
    """
    bounds = np.searchsorted(dst, np.arange(N_CORES + 1) * NODES_PER_CORE)
    counts = np.zeros((N_CORES, N_WIN, 2), dtype=np.int64)
    per_core_raw = []
    for k in range(N_CORES):
        e0, e1 = bounds[k], bounds[k + 1]
        dk = dst[e0:e1].astype(np.int64) - k * NODES_PER_CORE
        nk = nbr[e0:e1].astype(np.int64)
        w = dk >> 7
        q = (nk >= SPLIT).astype(np.int64)
        key = w * 2 + q
        np.add.at(counts[k], (w, q), 1)
        per_core_raw.append((e0, e1, dk, nk, key))

    T = np.maximum((counts.max(axis=0) + 127) // 128, 1)   # [N_WIN, 2]
    c0 = np.zeros((N_WIN, 2), dtype=np.int64)
    s = 0
    for w in range(N_WIN):
        for q in range(2):
            c0[w, q] = s
            s += T[w, q]
    S = int(s)

    per_core = []
    for k in range(N_CORES):
        e0, e1, dk, nk, key = per_core_raw[k]
        # sort slots by nbr within each group: ascending gather addresses
        order = np.lexsort((nk, key))
        sorted_key = key[order]
        group_start = np.searchsorted(sorted_key, np.arange(N_WIN * 2))
        j_within = np.arange(len(order)) - group_start[sorted_key]
        slot = np.empty(len(order), dtype=np.int64)
        slot[order] = j_within
        base_col = c0[key >> 1, key & 1]
        p = slot % 128
        c = base_col + slot // 128
        per_core.append((e0, e1, dk, nk, p, c))
    return T, c0, S, per_core


def _build_program(T, c0, S):
    nc = bacc.Bacc(num_swdge_queues=4)
    f32, bf16, i16 = mybir.dt.float32, mybir.dt.bfloat16, mybir.dt.int16
    i32 = mybir.dt.int32

    nf8 = nc.declare_dram_parameter("nf8", [N_NODES, H], F8, isOutput=False)
    identp = nc.declare_dram_parameter("ident", [128, 128], F8, isOutput=False)
    W16 = nc.declare_dram_parameter("W16", [H, H], bf16, isOutput=False)
    b16p = nc.declare_dram_parameter("b16", [1, H], bf16, isOutput=False)
    deg1 = nc.declare_dram_parameter("deg1", [1, N_WIN * WIN], bf16, isOutput=False)
    self8p = nc.declare_dram_parameter("self8", [128, N_WIN * H], F8, isOutput=False)
    ef = nc.declare_dram_parameter("ef", [128, S, H], F8, isOutput=False)
    ohp = nc.declare_dram_parameter("oh", [128, S * 128], F8, isOutput=False)
    cntp = nc.declare_dram_parameter("cnt", [1, 2 * N_WIN], i32, isOutput=False)
    gidx = nc.declare_dram_parameter("gidx", [128, 8 * S], i16, isOutput=False)
    outp = nc.declare_dram_parameter("out", [NODES_PER_CORE, H], f32, isOutput=True)

    twmax = int(T.max())
    LAG = 3

    with tile.TileContext(nc) as tc:
        nc.gpsimd.load_library(mlp)
        with (
            tc.tile_pool(name="const", bufs=1) as cpool,
            tc.tile_pool(name="psA", bufs=2, space="PSUM") as ppa,
            tc.tile_pool(name="psB", bufs=2, space="PSUM") as ppb,
            tc.tile_pool(name="psO", bufs=3, space="PSUM") as ppo,
            tc.tile_pool(name="gath", bufs=2 * LAG + 4) as gp,
            tc.tile_pool(name="ef8", bufs=LAG + 2) as e8p,
            tc.tile_pool(name="oneh", bufs=LAG + 2) as ohpool,
            tc.tile_pool(name="sT", bufs=3) as stp,
            tc.tile_pool(name="flush", bufs=3) as flp,
        ):
            # resident constants; gidx/cnt first so gathers can start early
            cnt_s = cpool.tile([1, 2 * N_WIN], i32)
            nc.scalar.dma_start(out=cnt_s[:], in_=cntp[:])
            gidx_s = cpool.tile([128, 8 * S], i16)
            third = ((8 * S) // 3) & ~7
            nc.sync.dma_start(out=gidx_s[:, :third], in_=gidx[:, :third])
            nc.sync.dma_start(out=gidx_s[:, third:2 * third], in_=gidx[:, third:2 * third])
            nc.sync.dma_start(out=gidx_s[:, 2 * third:], in_=gidx[:, 2 * third:])
            id8 = cpool.tile([128, 128], F8)
            nc.scalar.dma_start(out=id8[:], in_=identp[:])
            w16 = cpool.tile([128, 2 * H], bf16)           # W in two K-halves
            nc.scalar.dma_start(out=w16[:, :H], in_=W16[0:128, :])
            nc.scalar.dma_start(out=w16[:, H:], in_=W16[128:256, :])
            b16 = cpool.tile([1, H], bf16)
            nc.scalar.dma_start(out=b16[:], in_=b16p[:])
            d16 = cpool.tile([1, N_WIN * WIN], bf16)
            nc.scalar.dma_start(out=d16[:], in_=deg1[:])
            self_s = cpool.tile([128, N_WIN * H], F8)
            nc.scalar.dma_start(out=self_s[:], in_=self8p[:])

            # zero the gather pool once: count-trimmed pad slots must hold
            # finite fp8 values (the one-hot zero rows null them out, but
            # NaN * 0 would still poison the psum)
            for _ in range(2 * LAG + 4):
                z = gp.tile([128, twmax * H], F8, tag="gath")
                nc.vector.memset(z[:], 0.0)

            creg = nc.gpsimd.alloc_register("gather_cnt")
            g_tiles = {}
            eo_tiles = {}

            def issue_dmas(w):
                for q in range(2):
                    tw = int(T[w, q])
                    cc = int(c0[w, q])
                    g = gp.tile([128, twmax * H], F8, tag="gath")
                    g_tiles[(w, q)] = g
                    src = nf8[0:32768, :] if q == 0 else nf8[SPLIT:N_NODES, :]
                    nc.gpsimd.reg_load(creg, cnt_s[0:1, 2 * w + q:2 * w + q + 1])
                    nc.gpsimd.dma_gather(
                        out_ap=g[:, :tw * H].rearrange("p (c d) -> p c d", d=H),
                        in_ap=src,
                        idxs_ap=gidx_s[:, 8 * cc: 8 * (cc + tw)],
                        num_idxs=tw * 128,
                        num_idxs_reg=creg,
                        elem_size=H,
                        single_packet=False,
                        queue_num=(2 * w + q) % 4,
                    )
                twin = int(T[w, 0] + T[w, 1])
                cc = int(c0[w, 0])
                e8 = e8p.tile([128, twin * H], F8, tag="ef8")
                nc.sync.dma_start(
                    out=e8[:].rearrange("p (c d) -> p c d", d=H),
                    in_=ef[:, cc:cc + twin, :],
                )
                oh = ohpool.tile([128, twin * 128], F8, tag="oneh")
                nc.sync.dma_start(out=oh[:], in_=ohp[:, cc * 128:(cc + twin) * 128])
                eo_tiles[w] = (e8, oh)

            pend = {}

            def compute(w):
                n0 = w * WIN
                psa = ppa.tile([128, 128], f32, tag="psa")
                psb = ppb.tile([128, 128], f32, tag="psb")
                pso = ppo.tile([128, H], f32, tag="pso")
                e8, oh = eo_tiles.pop(w)
                # self term opens the psT groups; bias opens pso
                sl = self_s[:, w * H:(w + 1) * H]
                nc.tensor.matmul(psa[:], lhsT=sl[:, :128], rhs=id8[:], start=True, stop=False)
                nc.tensor.matmul(psb[:], lhsT=sl[:, 128:], rhs=id8[:], start=True, stop=False)
                nc.tensor.matmul(pso[:], lhsT=d16[:, n0:n0 + WIN], rhs=b16[:], start=True, stop=False)
                # edge features (ready early)
                twin = int(T[w, 0] + T[w, 1])
                for c in range(twin):
                    ohc = oh[:, c * 128:(c + 1) * 128]
                    nc.tensor.matmul(pso[:], lhsT=ohc, rhs=e8[:, c * H:(c + 1) * H],
                                     start=False, stop=False)
                # gathered neighbor features, transposed accumulation
                for q in range(2):
                    tw = int(T[w, q])
                    off = 0 if q == 0 else int(T[w, 0])
                    g = g_tiles.pop((w, q))
                    for c in range(tw):
                        ohc = oh[:, (off + c) * 128:(off + c + 1) * 128]
                        gc = g[:, c * H:(c + 1) * H]
                        last = q == 1 and c == tw - 1
                        nc.tensor.matmul(psa[:], lhsT=gc[:, :128], rhs=ohc,
                                         start=False, stop=last)
                        nc.tensor.matmul(psb[:], lhsT=gc[:, 128:], rhs=ohc,
                                         start=False, stop=last)
                pend[w] = (psa, psb, pso)

            def transform(w):
                psa, psb, pso = pend.pop(w)
                n0 = w * WIN
                nn = min(WIN, NODES_PER_CORE - n0)
                st = stp.tile([128, H], bf16, tag="sT")
                nc.vector.tensor_copy(out=st[:, :128], in_=psa[:])
                nc.vector.tensor_copy(out=st[:, 128:], in_=psb[:])
                nc.tensor.matmul(pso[:], lhsT=st[:, :128], rhs=w16[:, :H],
                                 start=False, stop=False)
                nc.tensor.matmul(pso[:], lhsT=st[:, 128:], rhs=w16[:, H:],
                                 start=False, stop=True)
                fl = flp.tile([128, H], f32, tag="flush")
                nc.scalar.copy(out=fl[:nn, :], in_=pso[:nn, :])
                nc.scalar.dma_start(out=outp[n0:n0 + nn, :], in_=fl[:nn, :])

            for step in range(N_WIN + LAG):
                if step < N_WIN:
                    issue_dmas(step)
                if step >= LAG:
                    w = step - LAG
                    compute(w)
                    if w > 0:
                        transform(w - 1)
            transform(N_WIN - 1)

    nc.compile()
    return nc


def kernel(node_feats, edge_feats, W, b, dst, nbr, eid):
    global _last_in_maps
    node_feats = np.ascontiguousarray(np.asarray(node_feats, dtype=np.float32))
    edge_feats = np.ascontiguousarray(np.asarray(edge_feats, dtype=np.float32))
    W = np.ascontiguousarray(np.asarray(W, dtype=np.float32))
    b = np.asarray(b, dtype=np.float32).reshape(1, H)
    dst = np.asarray(dst, dtype=np.int32)
    nbr = np.asarray(nbr, dtype=np.int32)
    eid = np.asarray(eid, dtype=np.int32)

    T, c0, S, per_core = _build_schedule(dst, nbr)

    key = (S, T.tobytes())
    if key not in _cache:
        _cache.clear()
        _cache[key] = _build_program(T, c0, S)
    nc = _cache[key]

    bf = ml_dtypes.bfloat16
    nf8_arr = node_feats.astype(F8NP)
    ident = np.zeros((128, 128), dtype=F8NP)
    np.fill_diagonal(ident, F8NP(1.0))
    W16 = W.astype(bf)
    b16 = b.astype(bf)

    in_maps = []
    for k in range(N_CORES):
        e0, e1, dk, nk, p, c = per_core[k]
        ef_arr = np.zeros((128, S, H), dtype=F8NP)
        ef_arr[p, c] = edge_feats[eid[e0:e1]].astype(F8NP)
        oh_arr = np.zeros((128, S, 128), dtype=F8NP)
        oh_arr[p, c, dk & 127] = F8NP(1.0)
        # gather indices: slot-within-group j = (c - group_base)*128 + p,
        # encoded at [j%16, 8*group_base + j//16]; unused trailing slots
        # stay -1 so the count-register gather skips them
        gidx_arr = np.full((16, 8 * S), -1, dtype=np.int16)
        w_arr = dk >> 7
        q_arr = (nk >= SPLIT).astype(np.int64)
        idx_val = np.where(q_arr == 0, nk, nk - SPLIT)
        base_col = c0[w_arr, q_arr]
        j = (c - base_col) * 128 + p
        gidx_arr[j % 16, 8 * base_col + j // 16] = idx_val.astype(np.int16)
        gidx_full = np.tile(gidx_arr, (8, 1))
        cnt_arr = np.zeros((1, 2 * N_WIN), dtype=np.int32)
        np.add.at(cnt_arr[0], w_arr * 2 + q_arr, 1)
        deg1_arr = np.zeros((1, N_WIN * WIN), dtype=np.float32)
        deg1_arr[0, :NODES_PER_CORE] = 1.0
        np.add.at(deg1_arr[0], dk, 1.0)
        # own-node fp8 rows: self8[p, w*H + f] = nf8 row of node w*128+p
        own8 = np.zeros((N_WIN * WIN, H), dtype=F8NP)
        own8[:NODES_PER_CORE] = nf8_arr[k * NODES_PER_CORE:(k + 1) * NODES_PER_CORE]
        self8_arr = np.ascontiguousarray(
            own8.reshape(N_WIN, WIN, H).transpose(1, 0, 2).reshape(WIN, N_WIN * H)
        )
        in_maps.append({
            "nf8": nf8_arr,
            "ident": ident,
            "W16": W16,
            "b16": b16,
            "deg1": deg1_arr.astype(bf),
            "self8": self8_arr,
            "ef": ef_arr,
            "oh": oh_arr.reshape(128, S * 128),
            "cnt": cnt_arr,
            "gidx": gidx_full,
        })

    _last_in_maps = in_maps
    res = run_bass_kernel_spmd(nc, in_maps, list(range(N_CORES)))
    out = np.concatenate([res.results[k]["out"] for k in range(N_CORES)], axis=0)
    return out
